# revision 21
# baseline (speedup 1.0000x reference)
"""GRU-residual trajectory kernel for Trainium2 (8 NeuronCores, data-parallel).

Reference semantics (PyTorch GRUCell math):
    h' = (1-u) * n + u * h
    r  = sigmoid(W_ih_r z + b_ih_r + W_hh_r h + b_hh_r)
    u  = sigmoid(W_ih_u z + b_ih_u + W_hh_u h + b_hh_u)
    n  = tanh(W_ih_n z + b_ih_n + r * (W_hh_n h + b_hh_n))
    z' = z + dt * (W_head h' + b_head)
repeated `steps` times; output traj = [z0, z1, ..., z_steps] per batch row.

Device mapping (per core, batch shard Bc=2048, feature-major layout):
  SBUF state XC [68, Bc]: rows 0-63 = h, rows 64-66 = z, row 67 = ones
  (z at a 32-aligned partition base for the ACT copy; biases ride the ones
  row; the update gate is negated so sigmoid gives u' = 1-u directly).
  Per step, per column chunk of 512 (pipelined, 16 steps per loop iter):
    G1  = W1.T  @ XC   -> [u'-preact ; r-preact]       (PE fp32, PSUM)
    HI  = W23.T @ XC   -> [i_n ; h_n]                  (PE fp32, PSUM)
    S   = sigmoid(G1)                                  (ACT, one table set)
    T1  = S[r] * HI[h_n] ; T1 += HI[i_n]               (DVE)
    n   = tanh(T1)                                     (ACT, bridges partition base)
    T3  = n - h ; T3 *= S[u'] ; h += T3                (GPSIMD, h in place)
    Z   = W5.T @ XC = z + dt*(W_head h' + b_head)      (PE fp32, exact identity)
    Z -> XC z-rows (ACT copy) -> DRAM out[t] (DMA)
  All matmuls stay fp32: float32r/bf16 moving data truncates to ~bf16 and
  the 2048-step recurrence integrates that bias to ~0.1 rel error.
"""

import sys

for p in ("/opt/trn_rl_repo",):
    if p not in sys.path:
        sys.path.insert(0, p)

import numpy as np

import concourse.bacc as bacc
import concourse.bass as bass
import concourse.mybir as mybir
from concourse.tile import TileContext
from concourse.bass_utils import run_bass_kernel_spmd

N_CORES = 8
B_FULL = 16384
BC = B_FULL // N_CORES  # 2048 per core
D = 3
H = 64
K = H + D + 1  # 68 state rows: h (0:64), z (64:67), ones (67)
STEPS = 2048
CHUNK = 512
N_CHUNKS = BC // CHUNK
UNROLL = 16

F32 = mybir.dt.float32
F32R = mybir.dt.float32r
SIG = mybir.ActivationFunctionType.Sigmoid
TANH = mybir.ActivationFunctionType.Tanh

_NC_CACHE = {}


def _build(steps: int):
    if steps in _NC_CACHE:
        return _NC_CACHE[steps]
    nc = bacc.Bacc(None, target_bir_lowering=False)

    xc0 = nc.dram_tensor("xc0", [K, BC], F32, kind="ExternalInput")
    w1 = nc.dram_tensor("w1", [K, 2 * H], F32, kind="ExternalInput")
    w23 = nc.dram_tensor("w23", [K, 2 * H], F32, kind="ExternalInput")
    w5 = nc.dram_tensor("w5", [K, D], F32, kind="ExternalInput")
    zs = nc.dram_tensor("zs", [steps * D, BC], F32, kind="ExternalOutput")

    with TileContext(nc) as tc:
        with (
            tc.tile_pool(name="state", bufs=1) as state_pool,
            tc.tile_pool(name="wpool", bufs=1) as wpool,
            tc.tile_pool(name="spool", bufs=4) as spool,
            tc.tile_pool(name="tpool", bufs=4) as tpool,
            tc.tile_pool(name="pg1", bufs=2, space="PSUM") as pg1,
            tc.tile_pool(name="phi", bufs=2, space="PSUM") as phi,
            tc.tile_pool(name="pz", bufs=2, space="PSUM") as pz,
        ):
            xc = state_pool.tile([K, BC], F32)
            w1_t = wpool.tile([K, 2 * H], F32, tag="w1")
            w23_t = wpool.tile([K, 2 * H], F32, tag="w23")
            w5_t = wpool.tile([K, D], F32, tag="w5")

            nc.sync.dma_start(w1_t[:], w1[:])
            nc.sync.dma_start(w23_t[:], w23[:])
            nc.sync.dma_start(w5_t[:], w5[:])
            nc.sync.dma_start(xc[:], xc0[:])  # h=0 | z0 | ones

            # Pre-load the ACT spline table set that covers sigmoid+tanh+copy
            # so the fixpoint pass doesn't re-load it every loop iteration.
            try:
                from concourse.hw_specs import get_activation_tables

                tabs = list(get_activation_tables(nc.m.arch).items())
                need = {SIG, TANH, mybir.ActivationFunctionType.Copy}
                set_id = next(
                    i for i, (_, fns) in enumerate(tabs) if need <= fns
                )
            except Exception:
                set_id = 2  # sigmoid_and_others
            nc.scalar.add_instruction(
                mybir.InstLoadActFuncSet(
                    name=nc.get_next_instruction_name(),
                    ins=[],
                    outs=[],
                    act_func_set_id=set_id,
                )
            )

            unroll = next(u for u in (UNROLL, 8, 4, 2, 1) if steps % u == 0)
            with tc.For_i(0, steps // unroll) as tu:
                for uu in range(unroll):
                    t = tu * unroll + uu
                    for c in range(N_CHUNKS):
                        cs = slice(c * CHUNK, (c + 1) * CHUNK)
                        xcs = xc[:, cs]

                        g1 = pg1.tile([2 * H, CHUNK], F32)
                        hi = phi.tile([2 * H, CHUNK], F32)
                        for mh in range(0, CHUNK, 512):
                            ms = slice(mh, mh + 512)
                            nc.tensor.matmul(
                                g1[:, ms], w1_t[:], xcs[:, ms], start=True, stop=True
                            )
                            nc.tensor.matmul(
                                hi[:, ms], w23_t[:], xcs[:, ms], start=True, stop=True
                            )

                        # s = [u' ; r] (u' rows 0:H base 0, r rows H:2H base 64)
                        s = spool.tile([2 * H, CHUNK], F32, tag="s")
                        nc.scalar.activation(s[:], g1[:], SIG)

                        # t1 lives at base partition 64 to match r
                        t1 = tpool.tile([2 * H, CHUNK], F32, tag="t1")
                        nc.vector.tensor_mul(
                            t1[H : 2 * H, :], s[H : 2 * H, :], hi[H : 2 * H, :]
                        )
                        nc.vector.tensor_add(
                            t1[H : 2 * H, :], t1[H : 2 * H, :], hi[0:H, :]
                        )
                        # tanh bridges base 64 -> base 0
                        n_t = tpool.tile([H, CHUNK], F32, tag="n")
                        nc.scalar.activation(n_t[:], t1[H : 2 * H, :], TANH)

                        t3 = tpool.tile([H, CHUNK], F32, tag="t3")
                        # h' = h + u' * (n - h)
                        nc.gpsimd.tensor_sub(t3[:], n_t[:], xc[0:H, cs])
                        nc.gpsimd.tensor_mul(t3[:], t3[:], s[0:H, :])
                        nc.gpsimd.tensor_add(xc[0:H, cs], xc[0:H, cs], t3[:])

                        z_p = pz.tile([D, CHUNK], F32)
                        for mh in range(0, CHUNK, 512):
                            ms = slice(mh, mh + 512)
                            nc.tensor.matmul(
                                z_p[:, ms],
                                w5_t[:],
                                xc[:, cs][:, ms],
                                start=True,
                                stop=True,
                            )
                        nc.scalar.copy(xc[H : H + D, cs], z_p[:])
                        nc.sync.dma_start(
                            zs[bass.ds(t * D, D), cs], xc[H : H + D, cs]
                        )

    nc.finalize()
    _NC_CACHE[steps] = nc
    return nc


def _pack_weights(dt, W_ih, W_hh, b_ih, b_hh, W_head, b_head):
    """Host-side packing of the fused stationary weight matrices."""
    W_ih = np.asarray(W_ih, np.float32)
    W_hh = np.asarray(W_hh, np.float32)
    b_ih = np.asarray(b_ih, np.float32)
    b_hh = np.asarray(b_hh, np.float32)
    W_head = np.asarray(W_head, np.float32)
    b_head = np.asarray(b_head, np.float32)
    dt = np.float32(dt)

    ZR = slice(H, H + D)  # z rows 64:67
    ONE = K - 1  # ones row 67

    w1 = np.zeros((K, 2 * H), np.float32)
    # u gate, negated -> cols 0:H gives sigmoid(-a_u) = 1-u = u'
    w1[0:H, 0:H] = -W_hh[H : 2 * H].T
    w1[ZR, 0:H] = -W_ih[H : 2 * H].T
    w1[ONE, 0:H] = -(b_ih[H : 2 * H] + b_hh[H : 2 * H])
    # r gate -> cols H:2H
    w1[0:H, H : 2 * H] = W_hh[0:H].T
    w1[ZR, H : 2 * H] = W_ih[0:H].T
    w1[ONE, H : 2 * H] = b_ih[0:H] + b_hh[0:H]

    w23 = np.zeros((K, 2 * H), np.float32)
    # i_n -> cols 0:H (z + bias only)
    w23[ZR, 0:H] = W_ih[2 * H : 3 * H].T
    w23[ONE, 0:H] = b_ih[2 * H : 3 * H]
    # h_n -> cols H:2H (h + bias only)
    w23[0:H, H : 2 * H] = W_hh[2 * H : 3 * H].T
    w23[ONE, H : 2 * H] = b_hh[2 * H : 3 * H]

    # w5 computes z' = z + dt*(W_head h' + b_head) in one fp32 matmul
    w5 = np.zeros((K, D), np.float32)
    w5[0:H, :] = dt * W_head.T
    w5[ZR, :] = np.eye(D, dtype=np.float32)
    w5[ONE, :] = dt * b_head
    return w1, w23, w5


def kernel(z0, dt, steps, W_ih, W_hh, b_ih, b_hh, W_head, b_head):
    z0 = np.asarray(z0, np.float32)
    steps = int(steps)
    B, d = z0.shape
    assert (B, d) == (B_FULL, D)
    w1, w23, w5 = _pack_weights(dt, W_ih, W_hh, b_ih, b_hh, W_head, b_head)

    nc = _build(steps)
    in_maps = []
    for c in range(N_CORES):
        z0c = z0[c * BC : (c + 1) * BC]  # [BC, 3]
        xc0 = np.zeros((K, BC), np.float32)
        xc0[H : H + D, :] = z0c.T
        xc0[K - 1, :] = 1.0
        in_maps.append({"xc0": xc0, "w1": w1, "w23": w23, "w5": w5})
    res = run_bass_kernel_spmd(nc, in_maps, core_ids=list(range(N_CORES)))

    outs = []
    for c in range(N_CORES):
        zs = res.results[c]["zs"].reshape(steps, D, BC)
        traj = np.empty((BC, steps + 1, D), np.float32)
        traj[:, 0, :] = z0[c * BC : (c + 1) * BC]
        traj[:, 1:, :] = zs.transpose(2, 0, 1)
        outs.append(traj)
    return np.concatenate(outs, axis=0)
